# revision 32
# baseline (speedup 1.0000x reference)
"""Trainium2 Bass kernel for MiniEq2Net (gnn_message_passing).

Math (validated against the jax reference in float64, rel err ~3e-7):

Per batch b (X = x[b], [n=256, d=16]) the first eq-layer's input channels are
diag(X[:,d]) and X[:,d] outer X[:,d], so layer 1 collapses to
    G1[s] = S(s) + c'_{s,i} (row-broadcast) + delta_ij a_{s,i}
with S(s) = X diag(wt_s) X^T (symmetric, one K=64 matmul per 4-row group in a
packed (a=i%4, s) x (j) layout), and the diagonal handled exactly via tiny
[32,256] side computations (dn/dg/Hdc).  Layer 2 + pooling becomes two K=128
block-diagonal channel-mix matmuls over relu'd H and H^T plus a fused
relu-accumulate, with the diagonal / rowsum / total-sum basis terms folded
into per-partition biases and a closed-form correction.

Sharding: pure data parallel, one batch element per NeuronCore (B=8, 8 cores).

v4 layout: 4 merged input DMAs (vs 12).  Phase A: H stream in 1-bank
[128,512] pair units (K=64 matmuls; per-group relu+accum, DVE-heavy with a
few early groups on Act) and HT stream in 2-bank [128,1024] quad tiles
(K=96 matmuls folding the c'_j column bias via replicated-identity rows;
one wide Act relu per quad).  Suffix computes -(rho+kappa) via negated
host-side weights (WB3neg/PWrepneg + host-folded hdc4 rowsum bias) so the
bias chain is two tiny matmuls + two Act ops.  Phase B drains in quad tiles:
DVE processes whole quads with a single wide [128,1024]
max(U, -rho-kappa)+accum op (relu(U+r) == max(U,-r)+r; the +r correction is
summed per quad on Pool from rhoka and added to the accumulator), while Act
processes the remaining quads as per-group plain biased relus whose row-sums
Pool reduces from SBUF.  Phase-B matmuls interleave into spare PE/PSUM
capacity.  Pooling + the tiny MLP head run on the host from the DMA'd-out
[128, 41] acc tile.
"""

import numpy as np

N = 256          # n (graph nodes)
D = 16           # input channel count
NH = 32          # hidden channels
A = 4            # row-packing factor: partition p = a*32+s, row i = 4*g+a
G = N // A       # 64 row-groups
B = 8            # batch == cores
NQ = G // 4      # 16 phase-B quads (4 groups each)
DVE_QUADS = list(range(11))        # drained by DVE as wide quads
ACT_QUADS = list(range(11, 16))    # drained by Act as 4 narrow groups each
NACT = 4 * len(ACT_QUADS)
# acc tile: cols 0:len(DVE_QUADS) per-quad sums (corrected), then NACT
# per-group sums, then 1 diag-correction col
C_ACC_A = len(DVE_QUADS)
C_DIAG = C_ACC_A + NACT
C_RHO = C_DIAG + 1
NCOL = C_RHO + G
F32 = np.float32

_PROG_CACHE = {}


def _reorder_ag(arr):
    """Permute the trailing i axis (len 256) into (a, g) order:
    out[..., a*G+g] = arr[..., 4*g+a]."""
    sh = arr.shape[:-1]
    return arr.reshape(*sh, G, A).swapaxes(-1, -2).reshape(*sh, N)


# ---------------------------------------------------------------- host side

def _percore_inputs(xb, W1, b1, W2, b2, D1, db1, D2, db2, D3, db3):
    """Small per-core operands, precomputed in float64, packed into 4 blobs."""
    import ml_dtypes
    bf16 = ml_dtypes.bfloat16
    X = xb.astype(np.float64)                      # [256, 16]
    n = float(N)
    sigma = X.sum(0)
    wt = W1[D:, :, 0] + W1[D:, :, 1]               # [16,32]
    alpha = W1[:D, :, 0] + W1[:D, :, 1] + W1[:D, :, 2]
    beta = W1[D:, :, 2]
    abias = alpha.T @ X.T + beta.T @ (X.T ** 2)    # [32,256]
    gamma = W1[:D, :, 3] / n + W1[D:, :, 3] * sigma[:, None] / n
    k = (W1[:D, :, 4].T @ (sigma / n**2)
         + W1[D:, :, 4].T @ (sigma**2 / n**2) + b1)
    cp = gamma.T @ X.T + k[:, None]                # [32,256]
    XT = X.T

    WtBD = np.zeros((A * D, 128))
    for a in range(A):
        WtBD[a * D:(a + 1) * D, a * NH:(a + 1) * NH] = wt
    Xr = X.reshape(G, A, D).transpose(1, 2, 0).reshape(A * D, G)
    Cpp = cp.reshape(NH, G, A).transpose(2, 0, 1).reshape(128, G)

    def blockdiag(M):
        out = np.zeros((128, 128))
        for a in range(A):
            out[a * NH:(a + 1) * NH, a * NH:(a + 1) * NH] = M
        return out

    I32r4 = np.tile(np.eye(NH), (1, A))
    # diagonal-channel side computations, all host-side ((a, g) col order)
    t0 = wt.T @ _reorder_ag(XT ** 2) + _reorder_ag(cp)     # [32, 256]
    dn = np.maximum(t0, 0.0)
    dg = np.maximum(t0 + _reorder_ag(abias), 0.0)
    hdc = dg - dn
    hdc4 = hdc.reshape(NH, A, G).transpose(1, 0, 2).reshape(128, G)
    W01 = W2[:, :, 0] + W2[:, :, 1]
    PWrep = (np.tile(np.eye(NH), (A, 1)) @ (W2[:, :, 4] / n**2)) @ I32r4
    b2rep = np.tile(b2, A)
    # fold the hdc4 part of kappa's rsum into the (negated) krep bias
    b2negc = -(b2rep + PWrep.T @ hdc4.sum(1))

    # ---- bmain (bf16, [128, 640]): WtBDh | XT4h | Xr,Cpp as f32 bytes
    bmain = np.zeros((128, 640), dtype=bf16)
    bmain[0:64, 0:128] = WtBD.astype(bf16)
    bmain[0:64, 128:384] = np.tile(XT, (A, 1)).astype(bf16)
    # f32 operands shipped through the bf16 blob: round to bf16 precision
    # (zero low mantissa bytes) so no 16-bit half looks like a NaN
    xr32 = np.ascontiguousarray(Xr.astype(np.float32))
    xr32 = (xr32.view(np.uint32) & np.uint32(0xFFFF0000)).view(np.float32)
    bmain[0:64, 384:512] = np.ascontiguousarray(xr32).view(np.uint16).view(bf16)
    cpp32 = np.ascontiguousarray(Cpp.astype(np.float32))
    cpp32 = (cpp32.view(np.uint32) & np.uint32(0xFFFF0000)).view(np.float32)
    bmain[:, 512:640] = np.ascontiguousarray(cpp32).view(np.uint16).view(bf16)
    # ---- bht (bf16, [96, 256]): rhs96 = [XT tiled 4x ; cp]
    bht = np.concatenate([np.tile(XT, (A, 1)), cp], axis=0).astype(bf16)
    bht = np.ascontiguousarray(bht)
    # ---- bi32 (bf16, [32, G*128]): replicated identity (HT lhsT rows 64:96)
    bi32 = np.ascontiguousarray(np.tile(I32r4, (1, G)).astype(bf16))
    # ---- bwr (f32 bits, device dtype f32r): WB0 | WB1
    bwr = np.zeros((128, 256), dtype=F32)
    bwr[:, 0:128] = blockdiag(W2[:, :, 0])
    bwr[:, 128:256] = blockdiag(W2[:, :, 1])
    # ---- blate (f32, [128, 833])
    blate = np.zeros((128, 833), dtype=F32)
    blate[:, 0:128] = -blockdiag(W2[:, :, 3] / n)          # WB3neg
    blate[:, 128:256] = -PWrep                             # PWrepneg
    blate[:, 256:320] = hdc4
    blate[:, 320:321] = b2negc[:, None]
    blate[0:32, 321:577] = W01.T @ hdc + W2[:, :, 2].T @ dg   # qsb
    blate[0:32, 577:833] = W01.T @ dn                         # u2sb
    return {'bmain': bmain, 'bht': bht, 'bi32': bi32, 'bwr': bwr,
            'blate': blate}


# -------------------------------------------------------------- device side

def build_program():
    if 'nc' in _PROG_CACHE:
        return _PROG_CACHE['nc']

    from contextlib import ExitStack
    import concourse.bacc as bacc
    import concourse.tile as tile
    from concourse import mybir

    f32 = mybir.dt.float32
    f32r = mybir.dt.float32r
    bf16 = mybir.dt.bfloat16
    AF = mybir.ActivationFunctionType
    ALU = mybir.AluOpType

    nc = bacc.Bacc(trn_type="TRN2", target_bir_lowering=False)
    dram = {
        'bmain': nc.dram_tensor('bmain', [128, 640], bf16, kind="ExternalInput"),
        'bht': nc.dram_tensor('bht', [96, 256], bf16, kind="ExternalInput"),
        'bi32': nc.dram_tensor('bi32', [32, G * 128], bf16,
                               kind="ExternalInput"),
        'bwr': nc.dram_tensor('bwr', [128, 256], f32r,
                              kind="ExternalInput"),
        'blate': nc.dram_tensor('blate', [128, 833], f32,
                                kind="ExternalInput"),
    }
    yout_d = nc.dram_tensor("yout", [128, NCOL], f32, kind="ExternalOutput")

    with tile.TileContext(nc) as tc:
        ctx = ExitStack()
        consts = ctx.enter_context(tc.tile_pool(name="consts", bufs=1))
        big = ctx.enter_context(tc.tile_pool(name="big", bufs=1))
        zero256 = big.tile([128, 256], f32, name="zero256")
        nc.vector.memset(zero256, 0.0)
        H4 = big.tile([128, G * N], f32r, name="H4")
        HT4 = big.tile([128, G * N], f32r, name="HT4")
        r4 = big.tile([128, G], f32, name="r4")
        acc = big.tile([128, NCOL], f32, name="acc")
        nc.vector.memset(acc[:, C_DIAG:C_DIAG + 1], 0.0)
        lhsT_all = big.tile([96, G, 128], bf16, name="lhsT_all")

        # pre-trigger the Relu/Identity act-table load during DMA dead-time
        dummyA = big.tile([1, 1], f32, name="dummyA")
        nc.vector.memset(dummyA, 0.0)
        nc.scalar.activation(out=dummyA, in_=dummyA, func=AF.Relu)

        # ---- 4 input DMAs (order = need order)
        bmain = consts.tile([128, 640], bf16, name="bmain")
        nc.default_dma_engine.dma_start(out=bmain, in_=dram['bmain'].ap())
        bht = consts.tile([96, 256], bf16, name="bht")
        nc.default_dma_engine.dma_start(out=bht, in_=dram['bht'].ap())
        nc.default_dma_engine.dma_start(out=lhsT_all[64:96, :, :],
                                        in_=dram['bi32'].ap())
        bwr = consts.tile([128, 256], f32r, name="bwr")
        nc.default_dma_engine.dma_start(out=bwr, in_=dram['bwr'].ap())
        blate = consts.tile([128, 833], f32, name="blate")
        nc.default_dma_engine.dma_start(out=blate, in_=dram['blate'].ap())

        wtbdh = bmain[0:64, 0:128]
        xt4r = bmain[0:64, 128:384]
        xrh = bmain[0:64, 384:512].bitcast(f32)    # [64, 64] f32 view
        cpp = bmain[:, 512:640].bitcast(f32)       # [128, 64] f32 view
        rhs96r = bht
        wb0r = bwr[:, 0:128]
        wb1r = bwr[:, 128:256]
        wb3neg = blate[:, 0:128]
        pwrepneg = blate[:, 128:256]
        hdc4 = blate[:, 256:320]
        b2negc = blate[:, 320:321]
        qsb = blate[0:32, 321:577]
        u2sb = blate[0:32, 577:833]

        small = ctx.enter_context(tc.tile_pool(name="small", bufs=1))
        scrapD_pool = ctx.enter_context(tc.tile_pool(name="scrapD", bufs=2))
        scrapA_pool = ctx.enter_context(tc.tile_pool(name="scrapA", bufs=4))

        def prep(g):
            nc.gpsimd.tensor_scalar(lhsT_all[0:64, g, :], wtbdh,
                                    xrh[:, g:g + 1], None, ALU.mult)

        # Preps interleave into the A-loop (lookahead) so each H-matmul only
        # trails the writes it actually needs in program order; the wb
        # converts slot in early (needed by the first B-matmul ~7us)
        PREP_AHEAD = 6
        # first four preps on DVE (4x bf16 mode, 94ns) -- DVE is idle during
        # the DMA window and this unblocks the first H-matmuls ~1us earlier
        for g in range(4):
            nc.vector.tensor_scalar(lhsT_all[0:64, g, :], wtbdh,
                                    xrh[:, g:g + 1], None, ALU.mult)
        for g in range(4, PREP_AHEAD * 2):
            prep(g)

        psPark_pool = ctx.enter_context(
            tc.tile_pool(name="psPark", bufs=1, space="PSUM"))
        psA_ctx = ExitStack()
        psH_pool = psA_ctx.enter_context(
            tc.tile_pool(name="psH", bufs=2, space="PSUM"))
        psHT_pool = psA_ctx.enter_context(
            tc.tile_pool(name="psHT", bufs=2, space="PSUM"))

        # PE warm-up: dummy matmuls while the blob DMAs land
        psW = psH_pool.tile([128, 512], f32, name="psH")
        for w in range(8):
            nc.tensor.matmul(psW[:, 256:320], lhsT=zero256[0:64, 0:128],
                             rhs=zero256[0:64, 0:64], start=True, stop=True,
                             skip_group_check=True)

        def bmm(ps, q):
            """4 phase-B matmuls for quad q into ps [128,1024]."""
            for half in range(2):
                sl = slice((4 * q + 2 * half) * N, (4 * q + 2 * half + 2) * N)
                dst = ps[:, half * 512:(half + 1) * 512]
                nc.tensor.matmul(dst, lhsT=wb0r, rhs=H4[:, sl],
                                 start=True, stop=False, skip_group_check=True)
                nc.tensor.matmul(dst, lhsT=wb1r, rhs=HT4[:, sl],
                                 start=False, stop=True, skip_group_check=True)

        # ---- Phase A: 32 H-pair units; HT quads every 2 units.
        # Act takes one H-relu from each of the first 9 pairs (it is
        # otherwise idle until the HT stream starts); DVE takes the rest.
        psb0 = None
        for u in range(G // 2):
            psh = psH_pool.tile([128, 512], f32, name="psH")
            for j in range(2):
                g = 2 * u + j
                nc.tensor.matmul(psh[:, j * N:(j + 1) * N],
                                 lhsT=lhsT_all[0:64, g, :], rhs=xt4r,
                                 start=(j == 0), stop=(j == 1),
                                 skip_group_check=True)
            if (u + PREP_AHEAD) * 2 < G:
                prep(2 * (u + PREP_AHEAD))
                prep(2 * (u + PREP_AHEAD) + 1)
            for j in range(2):
                g = 2 * u + j
                half = psh[:, j * N:(j + 1) * N]
                gs = slice(g * N, (g + 1) * N)
                # Act helps with one relu per pair only where it has slack:
                # before the HT stream starts (u<4) and after it ends (u>=26)
                if u == 31 or (j == 0 and u < 5):
                    nc.scalar.activation(out=H4[:, gs], in_=half,
                                         func=AF.Relu, bias=cpp[:, g:g + 1],
                                         accum_out=r4[:, g:g + 1])
                else:
                    nc.vector.scalar_tensor_tensor(
                        H4[:, gs], half, cpp[:, g:g + 1], zero256,
                        ALU.add, ALU.max, accum_out=r4[:, g:g + 1])
            if u % 2 == 1:
                q = (u - 1) // 2
                psht = psHT_pool.tile([128, 4 * N], f32, name="psHT")
                for j in range(4):
                    g = 4 * q + j
                    nc.tensor.matmul(psht[:, j * N:(j + 1) * N],
                                     lhsT=lhsT_all[0:96, g, :], rhs=rhs96r,
                                     start=(j % 2 == 0), stop=(j % 2 == 1),
                                     skip_group_check=True)
                nc.scalar.activation(out=HT4[:, 4 * q * N:(4 * q + 4) * N],
                                     in_=psht, func=AF.Relu)
                if q == 0:
                    psb0 = psPark_pool.tile([128, 1024], f32, name="psPark")
                    bmm(psb0, 0)

        # ---- suffix: -(rho+kappa) bias chain
        rsum = small.tile([128, 1], f32, name="rsum")
        nc.vector.tensor_reduce(out=rsum, in_=r4,
                                axis=mybir.AxisListType.X, op=ALU.add)
        r4hat = small.tile([128, G], f32, name="r4hat")
        nc.gpsimd.tensor_add(r4hat, r4, hdc4)
        psT = psH_pool.tile([128, 512], f32, name="psH")
        nc.tensor.matmul(psT[:, 0:1], lhsT=pwrepneg, rhs=rsum,
                         start=True, stop=True, skip_group_check=True)
        nc.tensor.matmul(psT[:, 256:256 + G], lhsT=wb3neg, rhs=r4hat,
                         start=True, stop=True, skip_group_check=True)
        # rhokaneg = psT2 + (PWrepneg.T rsum)[:,0] + b2negc in ONE DVE op:
        # (psT2 add psT[:,0:1]-scalar) add b2negc-broadcast
        rhokaneg = small.tile([128, G], f32, name="rhokaneg")
        nc.vector.scalar_tensor_tensor(
            rhokaneg, psT[:, 256:256 + G], psT[:, 0:1],
            b2negc.broadcast_to([128, G]), ALU.add, ALU.add)
        rhokapos = small.tile([128, G], f32, name="rhokapos")
        nc.gpsimd.tensor_scalar(rhokapos, rhokaneg, -1.0, None, ALU.mult)
        psA_ctx.close()

        psB2_pool = ctx.enter_context(
            tc.tile_pool(name="psB2", bufs=2, space="PSUM"))
        psB3_pool = ctx.enter_context(
            tc.tile_pool(name="psB3", bufs=1, space="PSUM"))

        # ---- corr path ((a,g) order) -- overlaps the phase-B drain
        def emit_corr_dma():
            rhokr = small.tile([32, 256], f32, name="rhokr")
            for a in range(A):
                nc.default_dma_engine.dma_start(
                    out=rhokr[:, a * G:(a + 1) * G],
                    in_=rhokapos[a * NH:(a + 1) * NH, :])
            return rhokr

        def emit_corr_pool(rhokr):
            uii = small.tile([32, 256], f32, name="uii")
            nc.gpsimd.tensor_add(uii, u2sb, rhokr)
            t3 = small.tile([32, 256], f32, name="t3")
            nc.gpsimd.tensor_add(t3, uii, qsb)
            return uii, t3

        def emit_corr_dve(uii, t3):
            scrapS = small.tile([32, 256], f32, name="scrapS")
            cA2 = small.tile([32, 1], f32, name="cA2")
            nc.vector.tensor_scalar(scrapS, t3, 0.0, None, ALU.max, ALU.add,
                                    accum_out=cA2)
            scrapS2 = small.tile([32, 256], f32, name="scrapS2")
            cB2 = small.tile([32, 1], f32, name="cB2")
            nc.vector.tensor_scalar(scrapS2, uii, 0.0, None, ALU.max, ALU.add,
                                    accum_out=cB2)
            nc.vector.tensor_sub(acc[0:32, C_DIAG:C_DIAG + 1], cA2, cB2)

        rhokr = emit_corr_dma()
        uii, t3 = emit_corr_pool(rhokr)

        # ---- Phase B drain.
        # PE feed order interleaves Act quads (early, so the Act+Pool narrow
        # chain isn't starved) with DVE quads.  DVE program order: its quads
        # ascending; Act: its groups ascending; Pool: reduces in Act order.
        feed = [0, 1, 11, 2, 3, 12, 4, 5, 13, 6, 7, 14, 8, 15, 9, 10]
        dve_done = 0
        corr_emitted = False
        for qi_f, q in enumerate(feed):
            if q == 0:
                ps = psb0
            else:
                pool_q = psB2_pool if qi_f % 2 == 0 else psB3_pool
                ps = pool_q.tile([128, 1024], f32, name="psB2")
                bmm(ps, q)
            if q in DVE_QUADS:
                qi = DVE_QUADS.index(q)
                scr = scrapD_pool.tile([128, 1024], f32, name="scrapD")
                nrb = rhokaneg[:, 4 * q:4 * q + 4].unsqueeze(2) \
                    .broadcast_to([128, 4, 256])
                nc.vector.scalar_tensor_tensor(
                    scr.rearrange("p (g j) -> p g j", g=4),
                    ps.rearrange("p (g j) -> p g j", g=4),
                    0.0, nrb, ALU.add, ALU.max, accum_out=acc[:, qi:qi + 1])
                dve_done += 1
                if dve_done == 5 and not corr_emitted:
                    emit_corr_dve(uii, t3)
                    corr_emitted = True
            else:
                ai = ACT_QUADS.index(q)
                for j in range(4):
                    g = 4 * q + j
                    scr = scrapA_pool.tile([128, 256], f32, name="scrapA")
                    nc.scalar.activation(out=scr, in_=ps[:, j * N:(j + 1) * N],
                                         func=AF.Relu,
                                         bias=rhokapos[:, g:g + 1],
                                         accum_out=acc[:, C_ACC_A + 4 * ai + j:
                                                       C_ACC_A + 4 * ai + j + 1])
        if not corr_emitted:
            emit_corr_dve(uii, t3)

        # rhokapos rides out in the acc tile; the +256*quad-sum(rho)
        # correction for the DVE-quad accumulators happens on the host
        nc.gpsimd.tensor_copy(acc[:, C_RHO:C_RHO + G], rhokapos)
        nc.default_dma_engine.dma_start(out=yout_d.ap(), in_=acc)

        ctx.close()

    nc.compile()
    _PROG_CACHE['nc'] = nc
    return nc


def make_in_maps(inputs):
    x = np.asarray(inputs['x'], dtype=F32)
    args = [np.asarray(inputs[k], dtype=np.float64) for k in
            ('W1', 'b1', 'W2', 'b2', 'D1', 'db1', 'D2', 'db2', 'D3', 'db3')]
    return [_percore_inputs(x[b], *args) for b in range(B)]


def finish_host(out, inputs):
    """Pooling + tiny MLP head on the host: out is the device's [128, NCOL]
    acc tile (col C_DIAG = diagonal correction rows 0:32; cols C_RHO:
    rhokapos, used to correct the DVE-quad max-trick accumulators)."""
    out64 = out.astype(np.float64)
    rho = out64[:, C_RHO:C_RHO + G]
    accred = (out64[:, 0:C_DIAG].sum(1)
              + N * rho[:, 0:4 * len(DVE_QUADS)].sum(1))    # [128]
    corr = out64[0:32, C_DIAG]
    p = np.maximum(accred.reshape(A, NH).sum(0) + corr, 0)  # [32]
    h = np.maximum(p @ inputs['D1'] + inputs['db1'], 0)
    h = np.maximum(h @ inputs['D2'] + inputs['db2'], 0)
    return (h @ inputs['D3'] + inputs['db3']).astype(F32)


def kernel(**inputs) -> np.ndarray:
    from concourse.bass_utils import run_bass_kernel_spmd
    nc = build_program()
    in_maps = make_in_maps(inputs)
    res = run_bass_kernel_spmd(nc, in_maps, core_ids=list(range(B))).results
    return np.stack([finish_host(np.asarray(res[b]['yout']), inputs)
                     for b in range(B)], axis=0).astype(F32)


# revision 34
# speedup vs baseline: 1.2438x; 1.2438x over previous
"""Trainium2 Bass kernel for MiniEq2Net (gnn_message_passing).

Math (validated against the jax reference in float64, rel err ~3e-7):

Per batch b (X = x[b], [n=256, d=16]) the first eq-layer's input channels are
diag(X[:,d]) and X[:,d] outer X[:,d], so layer 1 collapses to
    G1[s] = S(s) + c'_{s,i} (row-broadcast) + delta_ij a_{s,i}
with S(s) = X diag(wt_s) X^T (symmetric, one K=64 matmul per 4-row group in a
packed (a=i%4, s) x (j) layout), and the diagonal handled exactly via tiny
[32,256] side computations (dn/dg/Hdc).  Layer 2 + pooling becomes two K=128
block-diagonal channel-mix matmuls over relu'd H and H^T plus a fused
relu-accumulate, with the diagonal / rowsum / total-sum basis terms folded
into per-partition biases and a closed-form correction.

Sharding: pure data parallel, one batch element per NeuronCore (B=8, 8 cores).

v4 layout: 4 merged input DMAs (vs 12).  Phase A: H stream in 1-bank
[128,512] pair units (K=64 matmuls; per-group relu+accum, DVE-heavy with a
few early groups on Act) and HT stream in 2-bank [128,1024] quad tiles
(K=96 matmuls folding the c'_j column bias via replicated-identity rows;
one wide Act relu per quad).  Suffix computes -(rho+kappa) via negated
host-side weights (WB3neg/PWrepneg + host-folded hdc4 rowsum bias) so the
bias chain is two tiny matmuls + two Act ops.  Phase B drains in quad tiles:
DVE processes whole quads with a single wide [128,1024]
max(U, -rho-kappa)+accum op (relu(U+r) == max(U,-r)+r; the +r correction is
summed per quad on Pool from rhoka and added to the accumulator), while Act
processes the remaining quads as per-group plain biased relus whose row-sums
Pool reduces from SBUF.  Phase-B matmuls interleave into spare PE/PSUM
capacity.  Pooling + the tiny MLP head run on the host from the DMA'd-out
[128, 41] acc tile.
"""

import numpy as np

N = 256          # n (graph nodes)
D = 16           # input channel count
NH = 32          # hidden channels
A = 4            # row-packing factor: partition p = a*32+s, row i = 4*g+a
G = N // A       # 64 row-groups
B = 8            # batch == cores
NQ = G // 4      # 16 phase-B quads (4 groups each)
DVE_QUADS = list(range(11))        # drained by DVE as wide quads
ACT_QUADS = list(range(11, 16))    # drained by Act as 4 narrow groups each
NACT = 4 * len(ACT_QUADS)
# acc tile: cols 0:len(DVE_QUADS) per-quad sums (corrected), then NACT
# per-group sums, then 1 diag-correction col
C_ACC_A = len(DVE_QUADS)
C_RHO = C_ACC_A + NACT
NCOL = C_RHO + G
F32 = np.float32

_PROG_CACHE = {}


def _reorder_ag(arr):
    """Permute the trailing i axis (len 256) into (a, g) order:
    out[..., a*G+g] = arr[..., 4*g+a]."""
    sh = arr.shape[:-1]
    return arr.reshape(*sh, G, A).swapaxes(-1, -2).reshape(*sh, N)


# ---------------------------------------------------------------- host side

def _percore_inputs(xb, W1, b1, W2, b2, D1, db1, D2, db2, D3, db3):
    """Small per-core operands, precomputed in float64, packed into 4 blobs."""
    import ml_dtypes
    bf16 = ml_dtypes.bfloat16
    X = xb.astype(np.float64)                      # [256, 16]
    n = float(N)
    sigma = X.sum(0)
    wt = W1[D:, :, 0] + W1[D:, :, 1]               # [16,32]
    alpha = W1[:D, :, 0] + W1[:D, :, 1] + W1[:D, :, 2]
    beta = W1[D:, :, 2]
    abias = alpha.T @ X.T + beta.T @ (X.T ** 2)    # [32,256]
    gamma = W1[:D, :, 3] / n + W1[D:, :, 3] * sigma[:, None] / n
    k = (W1[:D, :, 4].T @ (sigma / n**2)
         + W1[D:, :, 4].T @ (sigma**2 / n**2) + b1)
    cp = gamma.T @ X.T + k[:, None]                # [32,256]
    XT = X.T

    WtBD = np.zeros((A * D, 128))
    for a in range(A):
        WtBD[a * D:(a + 1) * D, a * NH:(a + 1) * NH] = wt
    Xr = X.reshape(G, A, D).transpose(1, 2, 0).reshape(A * D, G)
    Cpp = cp.reshape(NH, G, A).transpose(2, 0, 1).reshape(128, G)

    def blockdiag(M):
        out = np.zeros((128, 128))
        for a in range(A):
            out[a * NH:(a + 1) * NH, a * NH:(a + 1) * NH] = M
        return out

    I32r4 = np.tile(np.eye(NH), (1, A))
    # diagonal-channel side computations, all host-side ((a, g) col order)
    t0 = wt.T @ _reorder_ag(XT ** 2) + _reorder_ag(cp)     # [32, 256]
    dn = np.maximum(t0, 0.0)
    dg = np.maximum(t0 + _reorder_ag(abias), 0.0)
    hdc = dg - dn
    hdc4 = hdc.reshape(NH, A, G).transpose(1, 0, 2).reshape(128, G)
    W01 = W2[:, :, 0] + W2[:, :, 1]
    PWrep = (np.tile(np.eye(NH), (A, 1)) @ (W2[:, :, 4] / n**2)) @ I32r4
    b2rep = np.tile(b2, A)
    # fold the hdc4 part of kappa's rsum into the (negated) krep bias
    b2negc = -(b2rep + PWrep.T @ hdc4.sum(1))

    # ---- bmain (bf16, [128, 640]): WtBDh | XT4h | Xr,Cpp as f32 bytes
    bmain = np.zeros((128, 640), dtype=bf16)
    bmain[0:64, 0:128] = WtBD.astype(bf16)
    bmain[0:64, 128:384] = np.tile(XT, (A, 1)).astype(bf16)
    # f32 operands shipped through the bf16 blob: round to bf16 precision
    # (zero low mantissa bytes) so no 16-bit half looks like a NaN
    xr32 = np.ascontiguousarray(Xr.astype(np.float32))
    xr32 = (xr32.view(np.uint32) & np.uint32(0xFFFF0000)).view(np.float32)
    bmain[0:64, 384:512] = np.ascontiguousarray(xr32).view(np.uint16).view(bf16)
    cpp32 = np.ascontiguousarray(Cpp.astype(np.float32))
    cpp32 = (cpp32.view(np.uint32) & np.uint32(0xFFFF0000)).view(np.float32)
    bmain[:, 512:640] = np.ascontiguousarray(cpp32).view(np.uint16).view(bf16)
    # ---- bht (bf16, [96, 256]): rhs96 = [XT tiled 4x ; cp]
    bht = np.concatenate([np.tile(XT, (A, 1)), cp], axis=0).astype(bf16)
    bht = np.ascontiguousarray(bht)
    # ---- bi32 (bf16, [32, G*128]): replicated identity (HT lhsT rows 64:96)
    bi32 = np.ascontiguousarray(np.tile(I32r4, (1, G)).astype(bf16))
    # ---- bwr (f32 bits, device dtype f32r): WB0 | WB1
    bwr = np.zeros((128, 256), dtype=F32)
    bwr[:, 0:128] = blockdiag(W2[:, :, 0])
    bwr[:, 128:256] = blockdiag(W2[:, :, 1])
    # ---- blate (f32, [128, 833])
    blate = np.zeros((128, 833), dtype=F32)
    blate[:, 0:128] = -blockdiag(W2[:, :, 3] / n)          # WB3neg
    blate[:, 128:256] = -PWrep                             # PWrepneg
    blate[:, 256:320] = hdc4
    blate[:, 320:321] = b2negc[:, None]
    blate[0:32, 321:577] = W01.T @ hdc + W2[:, :, 2].T @ dg   # qsb
    blate[0:32, 577:833] = W01.T @ dn                         # u2sb
    return {'bmain': bmain, 'bht': bht, 'bi32': bi32, 'bwr': bwr,
            'blate': blate}


# -------------------------------------------------------------- device side

def build_program():
    if 'nc' in _PROG_CACHE:
        return _PROG_CACHE['nc']

    from contextlib import ExitStack
    import concourse.bacc as bacc
    import concourse.tile as tile
    from concourse import mybir

    f32 = mybir.dt.float32
    f32r = mybir.dt.float32r
    bf16 = mybir.dt.bfloat16
    AF = mybir.ActivationFunctionType
    ALU = mybir.AluOpType

    nc = bacc.Bacc(trn_type="TRN2", target_bir_lowering=False)
    dram = {
        'bmain': nc.dram_tensor('bmain', [128, 640], bf16, kind="ExternalInput"),
        'bht': nc.dram_tensor('bht', [96, 256], bf16, kind="ExternalInput"),
        'bi32': nc.dram_tensor('bi32', [32, G * 128], bf16,
                               kind="ExternalInput"),
        'bwr': nc.dram_tensor('bwr', [128, 256], f32r,
                              kind="ExternalInput"),
        'blate': nc.dram_tensor('blate', [128, 833], f32,
                                kind="ExternalInput"),
    }
    yout_d = nc.dram_tensor("yout", [128, NCOL], f32, kind="ExternalOutput")

    with tile.TileContext(nc) as tc:
        ctx = ExitStack()
        consts = ctx.enter_context(tc.tile_pool(name="consts", bufs=1))
        big = ctx.enter_context(tc.tile_pool(name="big", bufs=1))
        zero256 = big.tile([128, 256], f32, name="zero256")
        nc.vector.memset(zero256, 0.0)
        H4 = big.tile([128, G * N], f32r, name="H4")
        HT4 = big.tile([128, G * N], f32r, name="HT4")
        r4 = big.tile([128, G], f32, name="r4")
        acc = big.tile([128, NCOL], f32, name="acc")
        lhsT_all = big.tile([96, G, 128], bf16, name="lhsT_all")

        # pre-trigger the Relu/Identity act-table load during DMA dead-time
        dummyA = big.tile([1, 1], f32, name="dummyA")
        nc.vector.memset(dummyA, 0.0)
        nc.scalar.activation(out=dummyA, in_=dummyA, func=AF.Relu)

        # ---- 4 input DMAs (order = need order)
        bmain = consts.tile([128, 640], bf16, name="bmain")
        nc.default_dma_engine.dma_start(out=bmain, in_=dram['bmain'].ap())
        bht = consts.tile([96, 256], bf16, name="bht")
        nc.default_dma_engine.dma_start(out=bht, in_=dram['bht'].ap())
        nc.default_dma_engine.dma_start(out=lhsT_all[64:96, :, :],
                                        in_=dram['bi32'].ap())
        bwr = consts.tile([128, 256], f32r, name="bwr")
        nc.default_dma_engine.dma_start(out=bwr, in_=dram['bwr'].ap())
        blate = consts.tile([128, 833], f32, name="blate")
        nc.default_dma_engine.dma_start(out=blate, in_=dram['blate'].ap())

        wtbdh = bmain[0:64, 0:128]
        xt4r = bmain[0:64, 128:384]
        xrh = bmain[0:64, 384:512].bitcast(f32)    # [64, 64] f32 view
        cpp = bmain[:, 512:640].bitcast(f32)       # [128, 64] f32 view
        rhs96r = bht
        wb0r = bwr[:, 0:128]
        wb1r = bwr[:, 128:256]
        wb3neg = blate[:, 0:128]
        pwrepneg = blate[:, 128:256]
        hdc4 = blate[:, 256:320]
        b2negc = blate[:, 320:321]
        qsb = blate[0:32, 321:577]
        u2sb = blate[0:32, 577:833]

        small = ctx.enter_context(tc.tile_pool(name="small", bufs=1))
        scrapD_pool = ctx.enter_context(tc.tile_pool(name="scrapD", bufs=2))
        scrapA_pool = ctx.enter_context(tc.tile_pool(name="scrapA", bufs=4))

        def prep(g):
            nc.gpsimd.tensor_scalar(lhsT_all[0:64, g, :], wtbdh,
                                    xrh[:, g:g + 1], None, ALU.mult)

        # Preps interleave into the A-loop (lookahead) so each H-matmul only
        # trails the writes it actually needs in program order; the wb
        # converts slot in early (needed by the first B-matmul ~7us)
        PREP_AHEAD = 6
        # first four preps on DVE (4x bf16 mode, 94ns) -- DVE is idle during
        # the DMA window and this unblocks the first H-matmuls ~1us earlier
        for g in range(4):
            nc.vector.tensor_scalar(lhsT_all[0:64, g, :], wtbdh,
                                    xrh[:, g:g + 1], None, ALU.mult)
        for g in range(4, PREP_AHEAD * 2):
            prep(g)

        psA_ctx = ExitStack()
        psH_pool = psA_ctx.enter_context(
            tc.tile_pool(name="psH", bufs=3, space="PSUM"))
        psHT_pool = psA_ctx.enter_context(
            tc.tile_pool(name="psHT", bufs=2, space="PSUM"))

        # PE warm-up: dummy matmuls while the blob DMAs land
        psW = psH_pool.tile([128, 512], f32, name="psH")
        for w in range(8):
            nc.tensor.matmul(psW[:, 256:320], lhsT=zero256[0:64, 0:128],
                             rhs=zero256[0:64, 0:64], start=True, stop=True,
                             skip_group_check=True)

        def bmm(ps, q):
            """4 phase-B matmuls for quad q into ps [128,1024]."""
            for half in range(2):
                sl = slice((4 * q + 2 * half) * N, (4 * q + 2 * half + 2) * N)
                dst = ps[:, half * 512:(half + 1) * 512]
                nc.tensor.matmul(dst, lhsT=wb0r, rhs=H4[:, sl],
                                 start=True, stop=False, skip_group_check=True)
                nc.tensor.matmul(dst, lhsT=wb1r, rhs=HT4[:, sl],
                                 start=False, stop=True, skip_group_check=True)

        # ---- Phase A: 32 H-pair units; HT quads every 2 units.
        # Act takes one H-relu from each of the first 9 pairs (it is
        # otherwise idle until the HT stream starts); DVE takes the rest.
        psb0 = None
        for u in range(G // 2):
            psh = psH_pool.tile([128, 512], f32, name="psH")
            for j in range(2):
                g = 2 * u + j
                nc.tensor.matmul(psh[:, j * N:(j + 1) * N],
                                 lhsT=lhsT_all[0:64, g, :], rhs=xt4r,
                                 start=(j == 0), stop=(j == 1),
                                 skip_group_check=True)
            if (u + PREP_AHEAD) * 2 < G:
                prep(2 * (u + PREP_AHEAD))
                prep(2 * (u + PREP_AHEAD) + 1)
            for j in range(2):
                g = 2 * u + j
                half = psh[:, j * N:(j + 1) * N]
                gs = slice(g * N, (g + 1) * N)
                # Act helps with one relu per pair only where it has slack:
                # before the HT stream starts (u<4) and after it ends (u>=26)
                if u == 31 or (j == 0 and u < 5):
                    nc.scalar.activation(out=H4[:, gs], in_=half,
                                         func=AF.Relu, bias=cpp[:, g:g + 1],
                                         accum_out=r4[:, g:g + 1])
                else:
                    nc.vector.scalar_tensor_tensor(
                        H4[:, gs], half, cpp[:, g:g + 1], zero256,
                        ALU.add, ALU.max, accum_out=r4[:, g:g + 1])
            if u % 2 == 1:
                q = (u - 1) // 2
                psht = psHT_pool.tile([128, 4 * N], f32, name="psHT")
                for j in range(4):
                    g = 4 * q + j
                    nc.tensor.matmul(psht[:, j * N:(j + 1) * N],
                                     lhsT=lhsT_all[0:96, g, :], rhs=rhs96r,
                                     start=(j % 2 == 0), stop=(j % 2 == 1),
                                     skip_group_check=True)
                nc.scalar.activation(out=HT4[:, 4 * q * N:(4 * q + 4) * N],
                                     in_=psht, func=AF.Relu)

        # ---- suffix: -(rho+kappa) bias chain
        rsum = small.tile([128, 1], f32, name="rsum")
        nc.vector.tensor_reduce(out=rsum, in_=r4,
                                axis=mybir.AxisListType.X, op=ALU.add)
        r4hat = small.tile([128, G], f32, name="r4hat")
        nc.gpsimd.tensor_add(r4hat, r4, hdc4)
        psT = psH_pool.tile([128, 512], f32, name="psH")
        nc.tensor.matmul(psT[:, 0:1], lhsT=pwrepneg, rhs=rsum,
                         start=True, stop=True, skip_group_check=True)
        nc.tensor.matmul(psT[:, 256:256 + G], lhsT=wb3neg, rhs=r4hat,
                         start=True, stop=True, skip_group_check=True)
        # rhokaneg = psT2 + (PWrepneg.T rsum)[:,0] + b2negc in ONE DVE op:
        # (psT2 add psT[:,0:1]-scalar) add b2negc-broadcast
        rhokaneg = small.tile([128, G], f32, name="rhokaneg")
        nc.vector.scalar_tensor_tensor(
            rhokaneg, psT[:, 256:256 + G], psT[:, 0:1],
            b2negc.broadcast_to([128, G]), ALU.add, ALU.add)
        rhokapos = small.tile([128, G], f32, name="rhokapos")
        nc.gpsimd.tensor_scalar(rhokapos, rhokaneg, -1.0, None, ALU.mult)
        psA_ctx.close()

        psB2_pool = ctx.enter_context(
            tc.tile_pool(name="psB2", bufs=2, space="PSUM"))
        psB3_pool = ctx.enter_context(
            tc.tile_pool(name="psB3", bufs=2, space="PSUM"))

        # (diagonal-correction path runs on the host from rhokapos)

        # ---- Phase B drain.
        # PE feed order interleaves Act quads (early, so the Act+Pool narrow
        # chain isn't starved) with DVE quads.  DVE program order: its quads
        # ascending; Act: its groups ascending; Pool: reduces in Act order.
        feed = [0, 1, 11, 2, 3, 12, 4, 5, 13, 6, 7, 14, 8, 15, 9, 10]
        for qi_f, q in enumerate(feed):
            pool_q = psB2_pool if qi_f % 2 == 0 else psB3_pool
            ps = pool_q.tile([128, 1024], f32, name="psB2")
            bmm(ps, q)
            if q in DVE_QUADS:
                qi = DVE_QUADS.index(q)
                scr = scrapD_pool.tile([128, 1024], f32, name="scrapD")
                nrb = rhokaneg[:, 4 * q:4 * q + 4].unsqueeze(2) \
                    .broadcast_to([128, 4, 256])
                nc.vector.scalar_tensor_tensor(
                    scr.rearrange("p (g j) -> p g j", g=4),
                    ps.rearrange("p (g j) -> p g j", g=4),
                    0.0, nrb, ALU.add, ALU.max, accum_out=acc[:, qi:qi + 1])
            else:
                ai = ACT_QUADS.index(q)
                for j in range(4):
                    g = 4 * q + j
                    scr = scrapA_pool.tile([128, 256], f32, name="scrapA")
                    nc.scalar.activation(out=scr, in_=ps[:, j * N:(j + 1) * N],
                                         func=AF.Relu,
                                         bias=rhokapos[:, g:g + 1],
                                         accum_out=acc[:, C_ACC_A + 4 * ai + j:
                                                       C_ACC_A + 4 * ai + j + 1])
        # rhokapos rides out in the acc tile; the +256*quad-sum(rho)
        # correction for the DVE-quad accumulators happens on the host
        nc.gpsimd.tensor_copy(acc[:, C_RHO:C_RHO + G], rhokapos)
        nc.default_dma_engine.dma_start(out=yout_d.ap(), in_=acc)

        ctx.close()

    nc.compile()
    _PROG_CACHE['nc'] = nc
    return nc


def make_in_maps(inputs):
    x = np.asarray(inputs['x'], dtype=F32)
    args = [np.asarray(inputs[k], dtype=np.float64) for k in
            ('W1', 'b1', 'W2', 'b2', 'D1', 'db1', 'D2', 'db2', 'D3', 'db3')]
    return [_percore_inputs(x[b], *args) for b in range(B)]


def finish_host(out, inputs, percore):
    """Pooling + tiny MLP head on the host: out is the device's [128, NCOL]
    acc tile; cols C_RHO: = rhokapos, which both corrects the DVE-quad
    max-trick accumulators and feeds the host-side diagonal correction."""
    out64 = out.astype(np.float64)
    rho = out64[:, C_RHO:C_RHO + G]
    accred = (out64[:, 0:C_RHO].sum(1)
              + N * rho[:, 0:4 * len(DVE_QUADS)].sum(1))    # [128]
    # diagonal correction from rhokapos + the host-known qsb/u2sb tables
    blate = percore['blate']
    qsb = blate[0:32, 321:577].astype(np.float64)
    u2sb = blate[0:32, 577:833].astype(np.float64)
    rhokr = rho.reshape(A, NH, G).transpose(1, 0, 2).reshape(NH, N)
    uii = u2sb + rhokr
    corr = (np.maximum(uii + qsb, 0) - np.maximum(uii, 0)).sum(1)
    p = np.maximum(accred.reshape(A, NH).sum(0) + corr, 0)  # [32]
    h = np.maximum(p @ inputs['D1'] + inputs['db1'], 0)
    h = np.maximum(h @ inputs['D2'] + inputs['db2'], 0)
    return (h @ inputs['D3'] + inputs['db3']).astype(F32)


def kernel(**inputs) -> np.ndarray:
    from concourse.bass_utils import run_bass_kernel_spmd
    nc = build_program()
    in_maps = make_in_maps(inputs)
    res = run_bass_kernel_spmd(nc, in_maps, core_ids=list(range(B))).results
    return np.stack([finish_host(np.asarray(res[b]['yout']), inputs,
                                 in_maps[b])
                     for b in range(B)], axis=0).astype(F32)


# revision 36
# speedup vs baseline: 1.2467x; 1.0023x over previous
"""Trainium2 Bass kernel for MiniEq2Net (gnn_message_passing).

Math (validated against the jax reference in float64, rel err ~3e-7):

Per batch b (X = x[b], [n=256, d=16]) the first eq-layer's input channels are
diag(X[:,d]) and X[:,d] outer X[:,d], so layer 1 collapses to
    G1[s] = S(s) + c'_{s,i} (row-broadcast) + delta_ij a_{s,i}
with S(s) = X diag(wt_s) X^T (symmetric, one K=64 matmul per 4-row group in a
packed (a=i%4, s) x (j) layout), and the diagonal handled exactly via tiny
[32,256] side computations (dn/dg/Hdc).  Layer 2 + pooling becomes two K=128
block-diagonal channel-mix matmuls over relu'd H and H^T plus a fused
relu-accumulate, with the diagonal / rowsum / total-sum basis terms folded
into per-partition biases and a closed-form correction.

Sharding: pure data parallel, one batch element per NeuronCore (B=8, 8 cores).

v4 layout: 4 merged input DMAs (vs 12).  Phase A: H stream in 1-bank
[128,512] pair units (K=64 matmuls; per-group relu+accum, DVE-heavy with a
few early groups on Act) and HT stream in 2-bank [128,1024] quad tiles
(K=96 matmuls folding the c'_j column bias via replicated-identity rows;
one wide Act relu per quad).  Suffix computes -(rho+kappa) via negated
host-side weights (WB3neg/PWrepneg + host-folded hdc4 rowsum bias) so the
bias chain is two tiny matmuls + two Act ops.  Phase B drains in quad tiles:
DVE processes whole quads with a single wide [128,1024]
max(U, -rho-kappa)+accum op (relu(U+r) == max(U,-r)+r; the +r correction is
summed per quad on Pool from rhoka and added to the accumulator), while Act
processes the remaining quads as per-group plain biased relus whose row-sums
Pool reduces from SBUF.  Phase-B matmuls interleave into spare PE/PSUM
capacity.  Pooling + the tiny MLP head run on the host from the DMA'd-out
[128, 41] acc tile.
"""

import numpy as np

N = 256          # n (graph nodes)
D = 16           # input channel count
NH = 32          # hidden channels
A = 4            # row-packing factor: partition p = a*32+s, row i = 4*g+a
G = N // A       # 64 row-groups
B = 8            # batch == cores
NQ = G // 4      # 16 phase-B quads (4 groups each)
DVE_QUADS = list(range(11))        # drained by DVE as wide quads
ACT_QUADS = list(range(11, 16))    # drained by Act as 4 narrow groups each
NACT = 4 * len(ACT_QUADS)
# acc tile: cols 0:len(DVE_QUADS) per-quad sums (corrected), then NACT
# per-group sums, then 1 diag-correction col
C_ACC_A = len(DVE_QUADS)
C_RHO = C_ACC_A + NACT
NCOL = C_RHO + G
F32 = np.float32

_PROG_CACHE = {}


def _reorder_ag(arr):
    """Permute the trailing i axis (len 256) into (a, g) order:
    out[..., a*G+g] = arr[..., 4*g+a]."""
    sh = arr.shape[:-1]
    return arr.reshape(*sh, G, A).swapaxes(-1, -2).reshape(*sh, N)


# ---------------------------------------------------------------- host side

def _percore_inputs(xb, W1, b1, W2, b2, D1, db1, D2, db2, D3, db3):
    """Small per-core operands, precomputed in float64, packed into 4 blobs."""
    import ml_dtypes
    bf16 = ml_dtypes.bfloat16
    X = xb.astype(np.float64)                      # [256, 16]
    n = float(N)
    sigma = X.sum(0)
    wt = W1[D:, :, 0] + W1[D:, :, 1]               # [16,32]
    alpha = W1[:D, :, 0] + W1[:D, :, 1] + W1[:D, :, 2]
    beta = W1[D:, :, 2]
    abias = alpha.T @ X.T + beta.T @ (X.T ** 2)    # [32,256]
    gamma = W1[:D, :, 3] / n + W1[D:, :, 3] * sigma[:, None] / n
    k = (W1[:D, :, 4].T @ (sigma / n**2)
         + W1[D:, :, 4].T @ (sigma**2 / n**2) + b1)
    cp = gamma.T @ X.T + k[:, None]                # [32,256]
    XT = X.T

    WtBD = np.zeros((A * D, 128))
    for a in range(A):
        WtBD[a * D:(a + 1) * D, a * NH:(a + 1) * NH] = wt
    Xr = X.reshape(G, A, D).transpose(1, 2, 0).reshape(A * D, G)
    Cpp = cp.reshape(NH, G, A).transpose(2, 0, 1).reshape(128, G)

    def blockdiag(M):
        out = np.zeros((128, 128))
        for a in range(A):
            out[a * NH:(a + 1) * NH, a * NH:(a + 1) * NH] = M
        return out

    I32r4 = np.tile(np.eye(NH), (1, A))
    # diagonal-channel side computations, all host-side ((a, g) col order)
    t0 = wt.T @ _reorder_ag(XT ** 2) + _reorder_ag(cp)     # [32, 256]
    dn = np.maximum(t0, 0.0)
    dg = np.maximum(t0 + _reorder_ag(abias), 0.0)
    hdc = dg - dn
    hdc4 = hdc.reshape(NH, A, G).transpose(1, 0, 2).reshape(128, G)
    W01 = W2[:, :, 0] + W2[:, :, 1]
    PWrep = (np.tile(np.eye(NH), (A, 1)) @ (W2[:, :, 4] / n**2)) @ I32r4
    b2rep = np.tile(b2, A)
    # fold the hdc4 part of kappa's rsum into the (negated) krep bias
    b2negc = -(b2rep + PWrep.T @ hdc4.sum(1))

    # ---- bmain (bf16, [128, 640]): WtBDh | XT4h | Xr,Cpp as f32 bytes
    bmain = np.zeros((128, 640), dtype=bf16)
    bmain[0:64, 0:128] = WtBD.astype(bf16)
    bmain[0:64, 128:384] = np.tile(XT, (A, 1)).astype(bf16)
    # f32 operands shipped through the bf16 blob: round to bf16 precision
    # (zero low mantissa bytes) so no 16-bit half looks like a NaN
    xr32 = np.ascontiguousarray(Xr.astype(np.float32))
    xr32 = (xr32.view(np.uint32) & np.uint32(0xFFFF0000)).view(np.float32)
    bmain[0:64, 384:512] = np.ascontiguousarray(xr32).view(np.uint16).view(bf16)
    cpp32 = np.ascontiguousarray(Cpp.astype(np.float32))
    cpp32 = (cpp32.view(np.uint32) & np.uint32(0xFFFF0000)).view(np.float32)
    bmain[:, 512:640] = np.ascontiguousarray(cpp32).view(np.uint16).view(bf16)
    # ---- bht (bf16, [96, 256]): rhs96 = [XT tiled 4x ; cp]
    bht = np.concatenate([np.tile(XT, (A, 1)), cp], axis=0).astype(bf16)
    bht = np.ascontiguousarray(bht)
    # ---- bi32 (bf16, [32, G*128]): replicated identity (HT lhsT rows 64:96)
    bi32 = np.ascontiguousarray(np.tile(I32r4, (1, G)).astype(bf16))
    # ---- bwr (f32 bits, device dtype f32r): WB0 | WB1
    bwr = np.zeros((128, 256), dtype=F32)
    bwr[:, 0:128] = blockdiag(W2[:, :, 0])
    bwr[:, 128:256] = blockdiag(W2[:, :, 1])
    # ---- blate (f32, [128, 833])
    blate = np.zeros((128, 833), dtype=F32)
    blate[:, 0:128] = -blockdiag(W2[:, :, 3] / n)          # WB3neg
    blate[:, 128:256] = -PWrep                             # PWrepneg
    blate[:, 256:320] = hdc4
    blate[:, 320:321] = b2negc[:, None]
    blate[0:32, 321:577] = W01.T @ hdc + W2[:, :, 2].T @ dg   # qsb
    blate[0:32, 577:833] = W01.T @ dn                         # u2sb
    return {'bmain': bmain, 'bht': bht, 'bi32': bi32, 'bwr': bwr,
            'blate': blate}


# -------------------------------------------------------------- device side

def build_program():
    if 'nc' in _PROG_CACHE:
        return _PROG_CACHE['nc']

    from contextlib import ExitStack
    import concourse.bacc as bacc
    import concourse.tile as tile
    from concourse import mybir

    f32 = mybir.dt.float32
    f32r = mybir.dt.float32r
    bf16 = mybir.dt.bfloat16
    AF = mybir.ActivationFunctionType
    ALU = mybir.AluOpType

    nc = bacc.Bacc(trn_type="TRN2", target_bir_lowering=False)
    dram = {
        'bmain': nc.dram_tensor('bmain', [128, 640], bf16, kind="ExternalInput"),
        'bht': nc.dram_tensor('bht', [96, 256], bf16, kind="ExternalInput"),
        'bi32': nc.dram_tensor('bi32', [32, G * 128], bf16,
                               kind="ExternalInput"),
        'bwr': nc.dram_tensor('bwr', [128, 256], f32r,
                              kind="ExternalInput"),
        'blate': nc.dram_tensor('blate', [128, 833], f32,
                                kind="ExternalInput"),
    }
    yout_d = nc.dram_tensor("yout", [128, NCOL], f32, kind="ExternalOutput")

    with tile.TileContext(nc) as tc:
        ctx = ExitStack()
        consts = ctx.enter_context(tc.tile_pool(name="consts", bufs=1))
        big = ctx.enter_context(tc.tile_pool(name="big", bufs=1))
        zero256 = big.tile([128, 256], f32, name="zero256")
        nc.vector.memset(zero256, 0.0)
        H4 = big.tile([128, G * N], f32r, name="H4")
        HT4 = big.tile([128, G * N], f32r, name="HT4")
        r4 = big.tile([128, G], f32, name="r4")
        acc = big.tile([128, NCOL], f32, name="acc")
        lhsT_all = big.tile([96, G, 128], bf16, name="lhsT_all")

        # pre-trigger the Relu/Identity act-table load during DMA dead-time
        dummyA = big.tile([1, 1], f32, name="dummyA")
        nc.vector.memset(dummyA, 0.0)
        nc.scalar.activation(out=dummyA, in_=dummyA, func=AF.Relu)

        # ---- 4 input DMAs (order = need order)
        bmain = consts.tile([128, 640], bf16, name="bmain")
        nc.default_dma_engine.dma_start(out=bmain, in_=dram['bmain'].ap())
        bht = consts.tile([96, 256], bf16, name="bht")
        nc.default_dma_engine.dma_start(out=bht, in_=dram['bht'].ap())
        nc.default_dma_engine.dma_start(out=lhsT_all[64:96, :, :],
                                        in_=dram['bi32'].ap())
        bwr = consts.tile([128, 256], f32r, name="bwr")
        nc.default_dma_engine.dma_start(out=bwr, in_=dram['bwr'].ap())
        blate = consts.tile([128, 833], f32, name="blate")
        nc.default_dma_engine.dma_start(out=blate, in_=dram['blate'].ap())

        wtbdh = bmain[0:64, 0:128]
        xt4r = bmain[0:64, 128:384]
        xrh = bmain[0:64, 384:512].bitcast(f32)    # [64, 64] f32 view
        cpp = bmain[:, 512:640].bitcast(f32)       # [128, 64] f32 view
        rhs96r = bht
        wb0r = bwr[:, 0:128]
        wb1r = bwr[:, 128:256]
        wb3neg = blate[:, 0:128]
        pwrepneg = blate[:, 128:256]
        hdc4 = blate[:, 256:320]
        b2negc = blate[:, 320:321]
        qsb = blate[0:32, 321:577]
        u2sb = blate[0:32, 577:833]

        small = ctx.enter_context(tc.tile_pool(name="small", bufs=1))
        scrapD_pool = ctx.enter_context(tc.tile_pool(name="scrapD", bufs=2))
        scrapA_pool = ctx.enter_context(tc.tile_pool(name="scrapA", bufs=4))

        def prep(g):
            nc.gpsimd.tensor_scalar(lhsT_all[0:64, g, :], wtbdh,
                                    xrh[:, g:g + 1], None, ALU.mult)

        # Preps interleave into the A-loop (lookahead) so each H-matmul only
        # trails the writes it actually needs in program order; the wb
        # converts slot in early (needed by the first B-matmul ~7us)
        PREP_AHEAD = 6
        # first four preps on DVE (4x bf16 mode, 94ns) -- DVE is idle during
        # the DMA window and this unblocks the first H-matmuls ~1us earlier
        for g in range(4):
            nc.vector.tensor_scalar(lhsT_all[0:64, g, :], wtbdh,
                                    xrh[:, g:g + 1], None, ALU.mult)
        for g in range(4, PREP_AHEAD * 2):
            prep(g)

        psA_ctx = ExitStack()
        psH_pool = psA_ctx.enter_context(
            tc.tile_pool(name="psH", bufs=3, space="PSUM"))
        psHT_pool = psA_ctx.enter_context(
            tc.tile_pool(name="psHT", bufs=2, space="PSUM"))

        # PE warm-up: dummy matmuls while the blob DMAs land
        psW = psH_pool.tile([128, 512], f32, name="psH")
        for w in range(8):
            nc.tensor.matmul(psW[:, 256:320], lhsT=zero256[0:64, 0:128],
                             rhs=zero256[0:64, 0:64], start=True, stop=True,
                             skip_group_check=True)

        def bmm(ps, q):
            """4 phase-B matmuls for quad q into ps [128,1024]."""
            for half in range(2):
                sl = slice((4 * q + 2 * half) * N, (4 * q + 2 * half + 2) * N)
                dst = ps[:, half * 512:(half + 1) * 512]
                nc.tensor.matmul(dst, lhsT=wb0r, rhs=H4[:, sl],
                                 start=True, stop=False, skip_group_check=True)
                nc.tensor.matmul(dst, lhsT=wb1r, rhs=HT4[:, sl],
                                 start=False, stop=True, skip_group_check=True)

        # ---- Phase A: 32 H-pair units; HT quads every 2 units.
        # Act takes one H-relu from each of the first 9 pairs (it is
        # otherwise idle until the HT stream starts); DVE takes the rest.
        psb0 = None
        for u in range(G // 2):
            psh = psH_pool.tile([128, 512], f32, name="psH")
            for j in range(2):
                g = 2 * u + j
                nc.tensor.matmul(psh[:, j * N:(j + 1) * N],
                                 lhsT=lhsT_all[0:64, g, :], rhs=xt4r,
                                 start=(j == 0), stop=(j == 1),
                                 skip_group_check=True)
            if (u + PREP_AHEAD) * 2 < G:
                prep(2 * (u + PREP_AHEAD))
                prep(2 * (u + PREP_AHEAD) + 1)
            for j in range(2):
                g = 2 * u + j
                half = psh[:, j * N:(j + 1) * N]
                gs = slice(g * N, (g + 1) * N)
                # Act helps with one relu per pair only where it has slack:
                # before the HT stream starts (u<4) and after it ends (u>=26)
                if u == 31 or (j == 0 and u < 5):
                    nc.scalar.activation(out=H4[:, gs], in_=half,
                                         func=AF.Relu, bias=cpp[:, g:g + 1],
                                         accum_out=r4[:, g:g + 1])
                else:
                    nc.vector.scalar_tensor_tensor(
                        H4[:, gs], half, cpp[:, g:g + 1], zero256,
                        ALU.add, ALU.max, accum_out=r4[:, g:g + 1])
            if u % 2 == 1:
                q = (u - 1) // 2
                psht = psHT_pool.tile([128, 4 * N], f32, name="psHT")
                for j in range(4):
                    g = 4 * q + j
                    nc.tensor.matmul(psht[:, j * N:(j + 1) * N],
                                     lhsT=lhsT_all[0:96, g, :], rhs=rhs96r,
                                     start=(j % 2 == 0), stop=(j % 2 == 1),
                                     skip_group_check=True)
                nc.scalar.activation(out=HT4[:, 4 * q * N:(4 * q + 4) * N],
                                     in_=psht, func=AF.Relu)

        # ---- suffix: -(rho+kappa) bias chain.  Groups 0:62 finish on DVE
        # ~0.6us before Act's pair-31 relus, so the reductions and the big
        # rho matmul run on that prefix first and the 62:64 tail follows.
        rsum62 = small.tile([128, 1], f32, name="rsum62")
        nc.vector.tensor_reduce(out=rsum62, in_=r4[:, 0:62],
                                axis=mybir.AxisListType.X, op=ALU.add)
        rsum = small.tile([128, 1], f32, name="rsum")
        nc.vector.scalar_tensor_tensor(rsum, r4[:, 62:63], rsum62,
                                       r4[:, 63:64], ALU.add, ALU.add)
        r4hat = small.tile([128, G], f32, name="r4hat")
        nc.gpsimd.tensor_add(r4hat[:, 0:62], r4[:, 0:62], hdc4[:, 0:62])
        nc.gpsimd.tensor_add(r4hat[:, 62:64], r4[:, 62:64], hdc4[:, 62:64])
        psT = psH_pool.tile([128, 512], f32, name="psH")
        nc.tensor.matmul(psT[:, 0:1], lhsT=pwrepneg, rhs=rsum,
                         start=True, stop=True, skip_group_check=True)
        nc.tensor.matmul(psT[:, 256:256 + 62], lhsT=wb3neg,
                         rhs=r4hat[:, 0:62],
                         start=True, stop=True, skip_group_check=True)
        nc.tensor.matmul(psT[:, 256 + 62:256 + G], lhsT=wb3neg,
                         rhs=r4hat[:, 62:64],
                         start=True, stop=True, skip_group_check=True)
        # rhokaneg = psT2 + (PWrepneg.T rsum)[:,0] + b2negc in ONE DVE op:
        # (psT2 add psT[:,0:1]-scalar) add b2negc-broadcast
        rhokaneg = small.tile([128, G], f32, name="rhokaneg")
        nc.vector.scalar_tensor_tensor(
            rhokaneg, psT[:, 256:256 + G], psT[:, 0:1],
            b2negc.broadcast_to([128, G]), ALU.add, ALU.add)
        rhokapos = small.tile([128, G], f32, name="rhokapos")
        nc.gpsimd.tensor_scalar(rhokapos, rhokaneg, -1.0, None, ALU.mult)
        psA_ctx.close()

        psB2_pool = ctx.enter_context(
            tc.tile_pool(name="psB2", bufs=2, space="PSUM"))
        psB3_pool = ctx.enter_context(
            tc.tile_pool(name="psB3", bufs=2, space="PSUM"))

        # (diagonal-correction path runs on the host from rhokapos)

        # ---- Phase B drain.
        # PE feed order interleaves Act quads (early, so the Act+Pool narrow
        # chain isn't starved) with DVE quads.  DVE program order: its quads
        # ascending; Act: its groups ascending; Pool: reduces in Act order.
        feed = [0, 1, 11, 2, 3, 12, 4, 5, 13, 6, 7, 14, 8, 15, 9, 10]
        for qi_f, q in enumerate(feed):
            pool_q = psB2_pool if qi_f % 2 == 0 else psB3_pool
            ps = pool_q.tile([128, 1024], f32, name="psB2")
            bmm(ps, q)
            if q in DVE_QUADS:
                qi = DVE_QUADS.index(q)
                scr = scrapD_pool.tile([128, 1024], f32, name="scrapD")
                nrb = rhokaneg[:, 4 * q:4 * q + 4].unsqueeze(2) \
                    .broadcast_to([128, 4, 256])
                nc.vector.scalar_tensor_tensor(
                    scr.rearrange("p (g j) -> p g j", g=4),
                    ps.rearrange("p (g j) -> p g j", g=4),
                    0.0, nrb, ALU.add, ALU.max, accum_out=acc[:, qi:qi + 1])
            else:
                ai = ACT_QUADS.index(q)
                for j in range(4):
                    g = 4 * q + j
                    scr = scrapA_pool.tile([128, 256], f32, name="scrapA")
                    nc.scalar.activation(out=scr, in_=ps[:, j * N:(j + 1) * N],
                                         func=AF.Relu,
                                         bias=rhokapos[:, g:g + 1],
                                         accum_out=acc[:, C_ACC_A + 4 * ai + j:
                                                       C_ACC_A + 4 * ai + j + 1])
        # rhokapos rides out in the acc tile; the +256*quad-sum(rho)
        # correction for the DVE-quad accumulators happens on the host
        nc.gpsimd.tensor_copy(acc[:, C_RHO:C_RHO + G], rhokapos)
        nc.default_dma_engine.dma_start(out=yout_d.ap(), in_=acc)

        ctx.close()

    nc.compile()
    _PROG_CACHE['nc'] = nc
    return nc


def make_in_maps(inputs):
    x = np.asarray(inputs['x'], dtype=F32)
    args = [np.asarray(inputs[k], dtype=np.float64) for k in
            ('W1', 'b1', 'W2', 'b2', 'D1', 'db1', 'D2', 'db2', 'D3', 'db3')]
    return [_percore_inputs(x[b], *args) for b in range(B)]


def finish_host(out, inputs, percore):
    """Pooling + tiny MLP head on the host: out is the device's [128, NCOL]
    acc tile; cols C_RHO: = rhokapos, which both corrects the DVE-quad
    max-trick accumulators and feeds the host-side diagonal correction."""
    out64 = out.astype(np.float64)
    rho = out64[:, C_RHO:C_RHO + G]
    accred = (out64[:, 0:C_RHO].sum(1)
              + N * rho[:, 0:4 * len(DVE_QUADS)].sum(1))    # [128]
    # diagonal correction from rhokapos + the host-known qsb/u2sb tables
    blate = percore['blate']
    qsb = blate[0:32, 321:577].astype(np.float64)
    u2sb = blate[0:32, 577:833].astype(np.float64)
    rhokr = rho.reshape(A, NH, G).transpose(1, 0, 2).reshape(NH, N)
    uii = u2sb + rhokr
    corr = (np.maximum(uii + qsb, 0) - np.maximum(uii, 0)).sum(1)
    p = np.maximum(accred.reshape(A, NH).sum(0) + corr, 0)  # [32]
    h = np.maximum(p @ inputs['D1'] + inputs['db1'], 0)
    h = np.maximum(h @ inputs['D2'] + inputs['db2'], 0)
    return (h @ inputs['D3'] + inputs['db3']).astype(F32)


def kernel(**inputs) -> np.ndarray:
    from concourse.bass_utils import run_bass_kernel_spmd
    nc = build_program()
    in_maps = make_in_maps(inputs)
    res = run_bass_kernel_spmd(nc, in_maps, core_ids=list(range(B))).results
    return np.stack([finish_host(np.asarray(res[b]['yout']), inputs,
                                 in_maps[b])
                     for b in range(B)], axis=0).astype(F32)
